# revision 3
# baseline (speedup 1.0000x reference)
"""AxialAttention TRN2 Bass kernel.

Shapes (hardcoded): x [B=4,T=16,C=256,H=64,W=64] fp32.
N = B*T*H = 4096 lines of [L=64, C=256]; heads=8, d=32.
Sharding: 64 (b,t) blocks -> 8 per core across 8 cores.

Per-core dataflow, per (b,t) block (xT = x[b,t] viewed [C=256, HW=4096],
features-on-partitions native layout):
  qkT  = w_qk^T @ xT          (fp32r MMs, N=512)   q pre-scaled by 1/sqrt(d)
  v    = xT^T @ w_v           (row-major v, lhsT = xT columns)
  per 8-line group, per psum bank b (heads b, b+4 stacked in partitions):
    scoresT[k,q] psum = biasT (via bias-MM: lhsT=bias_h, rhs=tiled-I)
                      + kT_h^T @ qT_h      (bf16, tile_position packed)
    probsT = exp(scoresT)  (ACT, -> bf16 sbuf)     [no max-sub: |s| small]
    Z      = Ez^T @ probsT (column sums, psum [8, 512])
    rz     = 1/Z           (DVE)
    rbc    = Ebc^T @ rz    (broadcast rz over d=32 partitions, psum)
    oT     = v_lh^T @ probsT_lh  (bf16 packed MMs, psum)
    oT_sb  = oT * rbc      (DVE, psum x psum -> sbuf)
    outT   = w_proj^T @ oT_sb + b'  (fp32r MMs; b' = b_v@w_proj + b_proj)
  b_k dropped (softmax shift-invariant); b_q folded into q evacuation bias;
  b_v folded into b'.
"""

import numpy as np

B, T, C, H, W = 4, 16, 256, 64, 64
HEADS, D = 8, 32
NBT = B * T            # 64 (b,t) blocks
NCORES = 8
BT_PER_CORE = NBT // NCORES  # 8
HW = H * W             # 4096 positions per block
L = W                  # 64
GRP = 8                # lines per attention group
NGRP = H // GRP        # 8 groups per block
GQ = GRP * L           # 512 free columns per group


def _build_bass():
    import concourse.bacc as bacc
    import concourse.mybir as mybir
    from concourse.tile import TileContext

    f32 = mybir.dt.float32
    f32r = mybir.dt.float32r
    bf16 = mybir.dt.bfloat16
    AF = mybir.ActivationFunctionType

    nc = bacc.Bacc("TRN2", target_bir_lowering=False, debug=False,
                   num_devices=NCORES)

    # DRAM I/O (per-core shard)
    x_d = nc.dram_tensor("x", [BT_PER_CORE, C, HW], f32r, kind="ExternalInput").ap()
    st_d = nc.dram_tensor("statics", [128, 3108], f32r, kind="ExternalInput").ap()
    out_d = nc.dram_tensor("out", [BT_PER_CORE, C, HW], f32, kind="ExternalOutput").ap()

    def r(ap):
        return ap

    with TileContext(nc) as tc:
        with (
            tc.tile_pool(name="static", bufs=1) as stat,
            tc.tile_pool(name="xt", bufs=2) as pxt,
            tc.tile_pool(name="qk", bufs=4) as pqk,
            tc.tile_pool(name="vsb", bufs=2) as pv,
            tc.tile_pool(name="probs", bufs=3) as ppr,
            tc.tile_pool(name="zsb", bufs=3) as pz,
            tc.tile_pool(name="osb", bufs=6) as po,
            tc.tile_pool(name="outsb", bufs=4) as pout,
            tc.tile_pool(name="psA", bufs=4, space="PSUM") as psA,
            tc.tile_pool(name="psZ", bufs=1, space="PSUM") as psZ,
            tc.tile_pool(name="psM", bufs=3, space="PSUM") as psM,
        ):
            # ---- static loads: one packed DMA ----
            st = stat.tile([128, 3108], f32r, tag="st", name="statics_sb")
            nc.sync.dma_start(out=st, in_=st_d)
            wqk = [st[:, 512 * i:512 * (i + 1)] for i in range(2)]
            wv = [st[:, 1024 + 256 * i:1024 + 256 * (i + 1)] for i in range(2)]
            wp = [[st[:, 1536 + 256 * i + 128 * j:1536 + 256 * i + 128 * (j + 1)]
                   for j in range(2)] for i in range(2)]
            bias_st = st[:, 2048:2304]
            i8t = st[:, 2304:2816]
            bq = st[:, 2816:2818].bitcast(f32)
            bp = st[:, 2818:2820].bitcast(f32)
            ebc = st[0:8, 2820:3076]
            ez = stat.tile([128, 32], bf16, tag="ez", name="ez")
            bias_bf = stat.tile([128, 256], bf16, tag="biasbf", name="bias_bf")
            i8_bf = stat.tile([128, 512], bf16, tag="i8bf", name="i8_bf")
            with nc.allow_low_precision(reason="exact 0/1 constants"):
                nc.vector.tensor_copy(ez, st[:, 3076:3108])
                nc.vector.tensor_copy(bias_bf, bias_st)
                nc.vector.tensor_copy(i8_bf, i8t)

            for bt in range(BT_PER_CORE):
                # ---- load xT ----
                xt = [pxt.tile([128, HW], f32r, tag="xt", name="xt") for _ in range(2)]
                for kc in range(2):
                    nc.sync.dma_start(out=xt[kc], in_=x_d[bt, 128 * kc:128 * (kc + 1), :])

                # ---- qk projection: qkT [512, 4096] -> bf16 sbuf ----
                # feature chunks: mc 0,1 = q (256), mc 2,3 = k (256)
                qkT = [pqk.tile([128, HW], bf16, tag="qkT", name="qkT") for _ in range(4)]
                for mc in range(4):
                    for nn in range(8):
                        ps = psM.tile([128, 512], f32, tag="mm", name="psmm")
                        for kc in range(2):
                            nc.tensor.matmul(
                                ps, r(wqk[kc][:, 128 * mc:128 * (mc + 1)]),
                                r(xt[kc][:, 512 * nn:512 * (nn + 1)]),
                                start=(kc == 0), stop=(kc == 1))
                        dst = qkT[mc][:, 512 * nn:512 * (nn + 1)]
                        if mc < 2:  # q: fold b_q (pre-scaled) per-partition
                            nc.scalar.activation(dst, ps, AF.Identity,
                                                 bias=bq[:, mc:mc + 1], scale=1.0)
                        else:       # k: plain copy (b_k dropped)
                            nc.vector.tensor_copy(dst, ps)

                # ---- v projection (row-major): v [4096, 256] bf16 ----
                # v_sb [128 = 2 lines, 32 chunks * 256]
                v_sb = pv.tile([128, 8192], bf16, tag="vsb", name="vsb")
                for pc in range(32):  # position chunks of 128 (2 lines)
                    ps = psM.tile([128, 256], f32, tag="mm", name="psmmv")
                    for kc in range(2):
                        nc.tensor.matmul(
                            ps, r(xt[kc][:, 128 * pc:128 * (pc + 1)]),
                            r(wv[kc]), start=(kc == 0), stop=(kc == 1))
                    if pc % 2 == 0:
                        nc.scalar.copy(v_sb[:, 256 * pc:256 * (pc + 1)], ps)
                    else:
                        nc.vector.tensor_copy(v_sb[:, 256 * pc:256 * (pc + 1)], ps)

                # partition-swapped v copy so attnv lhsT can start at 64*(h//4)
                v_sw = pv.tile([128, 8192], bf16, tag="vsw", name="vsw")
                nc.sync.dma_start(out=v_sw[0:64, :], in_=v_sb[64:128, :])
                nc.sync.dma_start(out=v_sw[64:128, :], in_=v_sb[0:64, :])

                # ---- attention per 8-line group ----
                for g in range(NGRP):
                    l0 = g * GRP
                    # scoresT psum: 4 banks, bank b = heads (b, b+4)
                    sps = [psA.tile([128, GQ], f32, tag="att", name="psatt") for _ in range(4)]
                    # bias seed MMs (fp32r, N=512)
                    for b in range(4):
                        for hh in range(2):  # h = b + 4*hh
                            nc.tensor.matmul(
                                sps[b][64 * hh:64 * (hh + 1), :],
                                bias_bf[64 * hh:64 * (hh + 1),
                                        64 * b:64 * (b + 1)],
                                i8_bf[64 * hh:64 * (hh + 1), :],
                                start=True, stop=False,
                                tile_position=(64 * hh, 64 * hh))
                    # scoresT accumulate: kT_h^T @ qT_h  (bf16)
                    for li in range(GRP):
                        l = l0 + li
                        for h in range(HEADS):
                            hc, hr = h // 4, h % 4
                            kt = qkT[2 + hc][32 * hr:32 * (hr + 1),
                                             64 * l:64 * (l + 1)]
                            qt = qkT[hc][32 * hr:32 * (hr + 1),
                                         64 * l:64 * (l + 1)]
                            nc.tensor.matmul(
                                sps[hr][64 * hc:64 * (hc + 1),
                                        64 * li:64 * (li + 1)],
                                kt, qt, start=False, stop=True,
                                tile_position=(32 * hr, 64 * hc))
                    # exp -> probsT bf16 sbuf [128, 4*512]
                    probs = ppr.tile([128, 4 * GQ], bf16, tag="probs", name="probs")
                    for b in range(4):
                        nc.scalar.activation(
                            probs[:, GQ * b:GQ * (b + 1)], sps[b], AF.Exp,
                            scale=1.0)
                    # Z: column sums -> psum_z [8, 512]
                    zps = psZ.tile([8, GQ], f32, tag="z", name="psz")
                    for b in range(4):
                        nc.tensor.matmul(
                            zps, ez[:, 8 * b:8 * (b + 1)],
                            probs[:, GQ * b:GQ * (b + 1)],
                            start=(b == 0), stop=(b == 3))
                    z_sb = pz.tile([8, GQ], f32r, tag="z", name="zsb")
                    with nc.allow_low_precision(reason="f32r bits are f32"):
                        nc.vector.reciprocal(z_sb, zps)
                    # broadcast recip over d=32 partitions: rbc [128, 512] x2
                    rbc = [psA.tile([128, GQ], f32, tag="att", name="psatt") for _ in range(2)]
                    rbc_sb = [po.tile([128, GQ], f32, tag="rbc", name="rbcsb")
                              for _ in range(2)]
                    for c in range(2):
                        nc.tensor.matmul(
                            rbc[c], r(ebc[:, 128 * c:128 * (c + 1)]),
                            r(z_sb), start=True, stop=True)
                        nc.scalar.copy(rbc_sb[c], rbc[c])
                    # attn @ v -> oT psum [128, 512] x2 (chunk c = heads 4c..4c+3)
                    ops = [psA.tile([128, GQ], f32, tag="att", name="psatt") for _ in range(2)]
                    for li in range(GRP):
                        l = l0 + li
                        vcol = 256 * (l // 2)
                        for h in range(HEADS):
                            hc, hr = h // 4, h % 4
                            vsrc = v_sb if (l % 2) == hc else v_sw
                            vt = vsrc[64 * hc:64 * (hc + 1),
                                      vcol + 32 * h:vcol + 32 * (h + 1)]
                            pt = probs[64 * hc:64 * (hc + 1),
                                       GQ * hr + 64 * li:GQ * hr + 64 * (li + 1)]
                            nc.tensor.matmul(
                                ops[hc][32 * hr:32 * (hr + 1),
                                        64 * li:64 * (li + 1)],
                                vt, pt, start=True, stop=True,
                                tile_position=(64 * hc, 32 * hr))
                    # oT * rbc -> sbuf f32
                    oT = [po.tile([128, GQ], f32r, tag="oT", name="oT") for _ in range(2)]
                    with nc.allow_low_precision(reason="f32r bits are f32"):
                        for c in range(2):
                            nc.vector.tensor_mul(oT[c], ops[c], rbc_sb[c])
                    # proj + bias -> out sbuf -> DRAM
                    for mc in range(2):
                        ps = psM.tile([128, GQ], f32, tag="mm", name="psproj")
                        for kc in range(2):
                            nc.tensor.matmul(ps, r(wp[kc][mc]), r(oT[kc]),
                                             start=(kc == 0), stop=(kc == 1))
                        osb = pout.tile([128, GQ], f32, tag="out", name="outsb")
                        nc.scalar.activation(osb, ps, AF.Identity,
                                             bias=bp[:, mc:mc + 1], scale=1.0)
                        nc.sync.dma_start(
                            out=out_d[bt, 128 * mc:128 * (mc + 1),
                                      GQ * g:GQ * (g + 1)],
                            in_=osb)
    nc.compile()
    return nc


def _host_inputs(x, relative_bias, w_qkv, b_qkv, w_proj, b_proj):
    import ml_dtypes
    scale = D ** -0.5
    wq = w_qkv[:, :C] * scale          # [256, 256]
    wk = w_qkv[:, C:2 * C]
    wv = w_qkv[:, 2 * C:]
    bqv = b_qkv[:C] * scale            # [256]
    bv = b_qkv[2 * C:]
    wqk_full = np.concatenate([wq, wk], axis=1)        # [256, 512]
    wqk = np.stack([wqk_full[:128], wqk_full[128:]]).astype(np.float32)
    wvs = np.stack([wv[:128], wv[128:]]).astype(np.float32)
    wp = np.zeros((2, 2, 128, 128), np.float32)
    for kc in range(2):
        for mc in range(2):
            wp[kc, mc] = w_proj[128 * kc:128 * (kc + 1),
                                128 * mc:128 * (mc + 1)]
    bq = np.stack([bqv[:128], bqv[128:]], axis=1).astype(np.float32)  # [128,2]
    bpv = bv @ w_proj + b_proj                                       # [256]
    bp = np.stack([bpv[:128], bpv[128:]], axis=1).astype(np.float32)
    bias_st = np.zeros((128, 256), np.float32)
    for h in range(HEADS):
        # lhsT = bias_h [q-contract, k-M]; head h -> rows 64*(h//4), cols 64*(h%4)
        bias_st[64 * (h // 4):64 * (h // 4) + 64,
                64 * (h % 4):64 * (h % 4) + 64] = relative_bias[h]
    i8h = np.tile(np.eye(64, dtype=np.float32), (1, 8))              # [64, 512]
    i8t = np.concatenate([i8h, i8h], axis=0)                         # [128, 512]
    ez = np.zeros((128, 32), np.float32)
    for b in range(4):
        ez[0:64, 8 * b + 2 * b] = 1      # head b    -> z row 2b
        ez[64:128, 8 * b + 2 * b + 1] = 1  # head b+4 -> z row 2b+1
    ebc = np.zeros((8, 256), np.float32)
    for c in range(2):
        for hr in range(4):
            h = 4 * c + hr
            zrow = 2 * (h % 4) + (h // 4)
            ebc[zrow, 128 * c + 32 * hr:128 * c + 32 * (hr + 1)] = 1.0
    st = np.zeros((128, 3108), np.float32)
    st[:, 0:512] = wqk[0]
    st[:, 512:1024] = wqk[1]
    st[:, 1024:1280] = wvs[0]
    st[:, 1280:1536] = wvs[1]
    for i in range(2):
        for j in range(2):
            st[:, 1536 + 256 * i + 128 * j:1536 + 256 * i + 128 * (j + 1)] = wp[i, j]
    st[:, 2048:2304] = bias_st
    st[:, 2304:2816] = i8t
    st[:, 2816:2818] = bq
    st[:, 2818:2820] = bp
    st[0:8, 2820:3076] = ebc
    st[:, 3076:3108] = ez.astype(np.float32)
    return dict(statics=st)


TRACE = False
LAST_RESULT = None


def kernel(x, relative_bias, w_qkv, b_qkv, w_proj, b_proj):
    import sys
    if '/opt/trn_rl_repo' not in sys.path:
        sys.path.insert(0, '/opt/trn_rl_repo')
    from concourse.bass_utils import run_bass_kernel_spmd

    x = np.asarray(x, np.float32)
    const = _host_inputs(np.asarray(x, np.float32),
                         np.asarray(relative_bias, np.float32),
                         np.asarray(w_qkv, np.float32),
                         np.asarray(b_qkv, np.float32),
                         np.asarray(w_proj, np.float32),
                         np.asarray(b_proj, np.float32))
    # x [B,T,C,H,W] -> [64, 256, 4096]
    xr = np.ascontiguousarray(x.reshape(NBT, C, HW))
    nc = _build_bass()
    in_maps = []
    for c in range(NCORES):
        m = dict(const)
        m["x"] = np.ascontiguousarray(xr[c * BT_PER_CORE:(c + 1) * BT_PER_CORE])
        in_maps.append(m)
    res = run_bass_kernel_spmd(nc, in_maps, list(range(NCORES)), trace=TRACE)
    global LAST_RESULT
    LAST_RESULT = res
    outs = res.results
    out = np.concatenate([o["out"].reshape(BT_PER_CORE, C, HW) for o in outs],
                         axis=0)
    return out.reshape(B, T, C, H, W).astype(np.float32)



# revision 4
# speedup vs baseline: 1.3900x; 1.3900x over previous
"""AxialAttention TRN2 Bass kernel (v2).

Shapes (hardcoded): x [B=4,T=16,C=256,H=64,W=64] fp32.
N = B*T*H = 4096 lines of [L=64, C=256]; heads=8, d=32.
Sharding: 64 (b,t) blocks -> 8 per core across 8 cores.

v2 highlights vs v1:
  - x shipped bf16 (half the input DMA); all matmuls bf16 -> FWL weight loads.
  - bias seeding via 4 concurrent 64x64-tile MMs per bank pair (quad trick,
    second bias static carries the partition-swapped copies).
  - Z accumulated for 4 groups into one [128,512] psum tile (32-row slots,
    rows zero-padded by the Ez stationary) -> ONE reciprocal per 4 groups.
  - Ebc broadcast MMs read batched z at 32-aligned partition offsets.

Per-core dataflow, per (b,t) block (xT = x[b,t] viewed [C=256, HW=4096]):
  qkT  = w_qk^T @ xT       (bf16 MMs, N=512)  q pre-scaled by 1/sqrt(d)
  v    = xT^T @ w_v        (row-major v, lhsT = xT columns)
  per 4-group batch, per group (8 lines):
    seed sps banks with biasT (quad-tile MMs), accumulate kT_h^T @ qT_h
    probsT = exp(scoresT)  (ACT -> bf16 sbuf)
    zall[32j:32j+32] += Ez4_b^T @ probsT  (b=0..3, rows 2b,2b+1 live)
  rz4 = 1/zall   (one DVE reciprocal per batch)
  per group: rbc = Ebc^T @ rz4 slice; oT = v^T @ probsT; oT*rbc -> bf16;
             out = w_proj^T @ oT + b'  (b' = b_v@w_proj + b_proj)
  b_k dropped (softmax shift-invariant); b_q folded into q evacuation bias.
"""

import numpy as np

B, T, C, H, W = 4, 16, 256, 64, 64
HEADS, D = 8, 32
NBT = B * T            # 64 (b,t) blocks
NCORES = 8
BT_PER_CORE = NBT // NCORES  # 8
HW = H * W             # 4096 positions per block
L = W                  # 64
GRP = 8                # lines per attention group
NGRP = H // GRP        # 8 groups per block
GQ = GRP * L           # 512 free columns per group

# bf16 statics column layout
ST16_WQK = 0          # [128, 1024]  wqk (q cols pre-scaled)
ST16_WV = 1024        # [128, 512]
ST16_WP = 1536        # [128, 512]   wp[kc][mc] 128-col blocks
ST16_BIASQ = 2048     # [128, 256]   quad bias (4 x 64-col slots)
ST16_I8 = 2304        # [128, 512]   tiled identity
ST16_EZ4 = 2816       # [128, 128]   4 banks x 32 cols
ST16_COLS = 2944
# f32r statics
ST32_EBC = 0          # [128, 256]   ebc rows at 32-offsets
ST32_BQ = 256         # [128, 2]
ST32_BP = 258         # [128, 2]
ST32_COLS = 260


def _build_bass():
    import concourse.bacc as bacc
    import concourse.mybir as mybir
    from concourse.tile import TileContext

    f32 = mybir.dt.float32
    f32r = mybir.dt.float32r
    bf16 = mybir.dt.bfloat16
    AF = mybir.ActivationFunctionType

    nc = bacc.Bacc("TRN2", target_bir_lowering=False, debug=False,
                   num_devices=NCORES)

    x_d = nc.dram_tensor("x", [BT_PER_CORE, C, HW], bf16, kind="ExternalInput").ap()
    st16_d = nc.dram_tensor("st16", [128, ST16_COLS], bf16, kind="ExternalInput").ap()
    st32_d = nc.dram_tensor("st32", [128, ST32_COLS], f32r, kind="ExternalInput").ap()
    out_d = nc.dram_tensor("out", [BT_PER_CORE, C, HW], f32, kind="ExternalOutput").ap()

    with TileContext(nc) as tc:
        with (
            tc.tile_pool(name="static", bufs=1) as stat,
            tc.tile_pool(name="xt", bufs=2) as pxt,
            tc.tile_pool(name="qk", bufs=4) as pqk,
            tc.tile_pool(name="vsb", bufs=2) as pv,
            tc.tile_pool(name="probs", bufs=5) as ppr,
            tc.tile_pool(name="zsb", bufs=2) as pz,
            tc.tile_pool(name="osb", bufs=4) as po,
            tc.tile_pool(name="outsb", bufs=4) as pout,
            tc.tile_pool(name="psA", bufs=4, space="PSUM") as psA,
            tc.tile_pool(name="psZ", bufs=1, space="PSUM") as psZ,
            tc.tile_pool(name="psM", bufs=3, space="PSUM") as psM,
        ):
            st16 = stat.tile([128, ST16_COLS], bf16, tag="st16", name="st16")
            st32 = stat.tile([128, ST32_COLS], f32r, tag="st32", name="st32")
            nc.sync.dma_start(out=st16, in_=st16_d)
            nc.sync.dma_start(out=st32, in_=st32_d)
            wqk = [st16[:, ST16_WQK + 512 * i:ST16_WQK + 512 * (i + 1)]
                   for i in range(2)]
            wv = [st16[:, ST16_WV + 256 * i:ST16_WV + 256 * (i + 1)]
                  for i in range(2)]
            wp = [[st16[:, ST16_WP + 256 * i + 128 * j:ST16_WP + 256 * i + 128 * (j + 1)]
                   for j in range(2)] for i in range(2)]
            biasQ = st16[:, ST16_BIASQ:ST16_BIASQ + 256]
            i8t = st16[:, ST16_I8:ST16_I8 + 512]
            ez4 = st16[:, ST16_EZ4:ST16_EZ4 + 128]
            ebc4 = st32[:, ST32_EBC:ST32_EBC + 256]
            bq = st32[:, ST32_BQ:ST32_BQ + 2].bitcast(f32)
            bp = st32[:, ST32_BP:ST32_BP + 2].bitcast(f32)

            for bt in range(BT_PER_CORE):
                # ---- load xT (bf16) ----
                xt = [pxt.tile([128, HW], bf16, tag="xt", name="xt") for _ in range(2)]
                for kc in range(2):
                    nc.sync.dma_start(out=xt[kc], in_=x_d[bt, 128 * kc:128 * (kc + 1), :])

                # ---- qk projection: qkT [512, 4096] -> bf16 sbuf ----
                qkT = [pqk.tile([128, HW], bf16, tag="qkT", name="qkT") for _ in range(4)]
                for mc in range(4):
                    for nn in range(8):
                        ps = psM.tile([128, 512], f32, tag="mm", name="psmm")
                        for kc in range(2):
                            nc.tensor.matmul(
                                ps, wqk[kc][:, 128 * mc:128 * (mc + 1)],
                                xt[kc][:, 512 * nn:512 * (nn + 1)],
                                start=(kc == 0), stop=(kc == 1))
                        dst = qkT[mc][:, 512 * nn:512 * (nn + 1)]
                        if mc < 2:  # q: fold b_q (pre-scaled) per-partition
                            nc.scalar.activation(dst, ps, AF.Identity,
                                                 bias=bq[:, mc:mc + 1], scale=1.0)
                        else:       # k: plain copy (b_k dropped)
                            nc.vector.tensor_copy(dst, ps)

                # ---- v projection (row-major): v [4096, 256] bf16 ----
                v_sb = pv.tile([128, 8192], bf16, tag="vsb", name="vsb")
                for pc in range(32):  # position chunks of 128 (2 lines)
                    ps = psM.tile([128, 256], f32, tag="mm", name="psmmv")
                    for kc in range(2):
                        nc.tensor.matmul(
                            ps, xt[kc][:, 128 * pc:128 * (pc + 1)],
                            wv[kc], start=(kc == 0), stop=(kc == 1))
                    if pc % 2 == 0:
                        nc.scalar.copy(v_sb[:, 256 * pc:256 * (pc + 1)], ps)
                    else:
                        nc.vector.tensor_copy(v_sb[:, 256 * pc:256 * (pc + 1)], ps)

                # partition-swapped v copy so attnv lhsT can start at 64*(h//4)
                v_sw = pv.tile([128, 8192], bf16, tag="vsw", name="vsw")
                nc.sync.dma_start(out=v_sw[0:64, :], in_=v_sb[64:128, :])
                nc.sync.dma_start(out=v_sw[64:128, :], in_=v_sb[0:64, :])

                # ---- attention: two 4-group batches ----
                for half in range(2):
                    zall = psZ.tile([128, GQ], f32, tag="z", name="psz")
                    probs_t = []
                    for j in range(4):
                        g = 4 * half + j
                        l0 = g * GRP
                        sps = [psA.tile([128, GQ], f32, tag="att", name="psatt")
                               for _ in range(4)]
                        # bias seed: 2 quads x 4 concurrent 64x64-tile MMs
                        for qd in range(2):  # banks (2qd, 2qd+1)
                            s0 = 128 * qd
                            nc.tensor.matmul(  # T0: bank 2qd, heads 2qd
                                sps[2 * qd][0:64, :], biasQ[0:64, s0:s0 + 64],
                                i8t[0:64, :], start=True, stop=False,
                                tile_position=(0, 0))
                            nc.tensor.matmul(  # T10: bank 2qd, head 2qd+4
                                sps[2 * qd][64:128, :], biasQ[64:128, s0:s0 + 64],
                                i8t[64:128, :], start=True, stop=False,
                                tile_position=(64, 64))
                            nc.tensor.matmul(  # T8: bank 2qd+1, head 2qd+1
                                sps[2 * qd + 1][0:64, :],
                                biasQ[64:128, s0 + 64:s0 + 128],
                                i8t[64:128, :], start=True, stop=False,
                                tile_position=(64, 0))
                            nc.tensor.matmul(  # T2: bank 2qd+1, head 2qd+5
                                sps[2 * qd + 1][64:128, :],
                                biasQ[0:64, s0 + 64:s0 + 128],
                                i8t[0:64, :], start=True, stop=False,
                                tile_position=(0, 64))
                        # scoresT accumulate: kT_h^T @ qT_h
                        for li in range(GRP):
                            l = l0 + li
                            for h in range(HEADS):
                                hc, hr = h // 4, h % 4
                                kt = qkT[2 + hc][32 * hr:32 * (hr + 1),
                                                 64 * l:64 * (l + 1)]
                                qt = qkT[hc][32 * hr:32 * (hr + 1),
                                             64 * l:64 * (l + 1)]
                                nc.tensor.matmul(
                                    sps[hr][64 * hc:64 * (hc + 1),
                                            64 * li:64 * (li + 1)],
                                    kt, qt, start=False, stop=True,
                                    tile_position=(32 * hr, 64 * hc))
                        # exp -> probsT bf16 sbuf [128, 4*512]
                        probs = ppr.tile([128, 4 * GQ], bf16, tag="probs", name="probs")
                        probs_t.append(probs)
                        for b in range(4):
                            nc.scalar.activation(
                                probs[:, GQ * b:GQ * (b + 1)], sps[b], AF.Exp,
                                scale=1.0)
                        # Z: column sums -> zall rows 32j..32j+32 (2b,2b+1 live)
                        for b in range(4):
                            nc.tensor.matmul(
                                zall[32 * j:32 * (j + 1), :],
                                ez4[:, 32 * b:32 * (b + 1)],
                                probs[:, GQ * b:GQ * (b + 1)],
                                start=(b == 0), stop=(b == 3),
                                tile_position=(0, 32 * j))
                    # one reciprocal for the whole batch
                    z4 = pz.tile([128, GQ], f32r, tag="z", name="z4sb")
                    with nc.allow_low_precision(reason="f32r bits are f32"):
                        nc.vector.reciprocal(z4, zall)
                    # phase 2 per group
                    for j in range(4):
                        g = 4 * half + j
                        l0 = g * GRP
                        probs = probs_t[j]
                        rbc = [psA.tile([128, GQ], f32, tag="att", name="psatt")
                               for _ in range(2)]
                        rbc_sb = [po.tile([128, GQ], f32, tag="rbc", name="rbcsb")
                                  for _ in range(2)]
                        for c in range(2):
                            nc.tensor.matmul(
                                rbc[c], ebc4[32 * j:32 * j + 8, 128 * c:128 * (c + 1)],
                                z4[32 * j:32 * j + 8, :], start=True, stop=True,
                                tile_position=(32 * j, 0))
                            nc.scalar.copy(rbc_sb[c], rbc[c])
                        ops = [psA.tile([128, GQ], f32, tag="att", name="psatt")
                               for _ in range(2)]
                        for li in range(GRP):
                            l = l0 + li
                            vcol = 256 * (l // 2)
                            for h in range(HEADS):
                                hc, hr = h // 4, h % 4
                                vsrc = v_sb if (l % 2) == hc else v_sw
                                vt = vsrc[64 * hc:64 * (hc + 1),
                                          vcol + 32 * h:vcol + 32 * (h + 1)]
                                pt = probs[64 * hc:64 * (hc + 1),
                                           GQ * hr + 64 * li:GQ * hr + 64 * (li + 1)]
                                nc.tensor.matmul(
                                    ops[hc][32 * hr:32 * (hr + 1),
                                            64 * li:64 * (li + 1)],
                                    vt, pt, start=True, stop=True,
                                    tile_position=(64 * hc, 32 * hr))
                        # oT * rbc -> bf16 sbuf
                        oT = [po.tile([128, GQ], bf16, tag="oT", name="oT")
                              for _ in range(2)]
                        with nc.allow_low_precision(reason="attn probs tolerate bf16"):
                            for c in range(2):
                                nc.vector.tensor_mul(oT[c], ops[c], rbc_sb[c])
                        # proj + bias -> out sbuf -> DRAM
                        for mc in range(2):
                            ps = psM.tile([128, GQ], f32, tag="mm", name="psproj")
                            for kc in range(2):
                                nc.tensor.matmul(ps, wp[kc][mc], oT[kc],
                                                 start=(kc == 0), stop=(kc == 1))
                            osb = pout.tile([128, GQ], f32, tag="out", name="outsb")
                            nc.scalar.activation(osb, ps, AF.Identity,
                                                 bias=bp[:, mc:mc + 1], scale=1.0)
                            nc.sync.dma_start(
                                out=out_d[bt, 128 * mc:128 * (mc + 1),
                                          GQ * g:GQ * (g + 1)],
                                in_=osb)
    nc.compile()
    return nc


def _host_inputs(x, relative_bias, w_qkv, b_qkv, w_proj, b_proj):
    import ml_dtypes
    bfloat16 = ml_dtypes.bfloat16
    scale = D ** -0.5
    wq = w_qkv[:, :C] * scale
    wk = w_qkv[:, C:2 * C]
    wvm = w_qkv[:, 2 * C:]
    bqv = b_qkv[:C] * scale
    bv = b_qkv[2 * C:]

    st16 = np.zeros((128, ST16_COLS), np.float32)
    wqk_full = np.concatenate([wq, wk], axis=1)          # [256, 512]
    st16[:, ST16_WQK:ST16_WQK + 512] = wqk_full[:128]
    st16[:, ST16_WQK + 512:ST16_WQK + 1024] = wqk_full[128:]
    st16[:, ST16_WV:ST16_WV + 256] = wvm[:128]
    st16[:, ST16_WV + 256:ST16_WV + 512] = wvm[128:]
    for kc in range(2):
        for mc in range(2):
            st16[:, ST16_WP + 256 * kc + 128 * mc:ST16_WP + 256 * kc + 128 * (mc + 1)] = \
                w_proj[128 * kc:128 * (kc + 1), 128 * mc:128 * (mc + 1)]
    # quad bias: slot s (64 cols): quad qd=s//2; see kernel seed MMs
    #  s=0: parts 0-63 = bias_0, parts 64-127 = bias_4   (T0 / T10 of quad 0)
    #  s=1: parts 0-63 = bias_5, parts 64-127 = bias_1   (T2 / T8  of quad 0)
    #  s=2: parts 0-63 = bias_2, parts 64-127 = bias_6   (T0 / T10 of quad 1)
    #  s=3: parts 0-63 = bias_7, parts 64-127 = bias_3   (T2 / T8  of quad 1)
    bq_layout = [(0, 4), (5, 1), (2, 6), (7, 3)]
    for s, (lo, hi) in enumerate(bq_layout):
        st16[0:64, ST16_BIASQ + 64 * s:ST16_BIASQ + 64 * (s + 1)] = relative_bias[lo]
        st16[64:128, ST16_BIASQ + 64 * s:ST16_BIASQ + 64 * (s + 1)] = relative_bias[hi]
    i8h = np.tile(np.eye(64, dtype=np.float32), (1, 8))  # [64, 512]
    st16[:, ST16_I8:ST16_I8 + 512] = np.concatenate([i8h, i8h], axis=0)
    # ez4: bank b cols [32b..32b+32]: col 2b -> sum parts 0-63, col 2b+1 -> 64-127
    ez4 = np.zeros((128, 128), np.float32)
    for b in range(4):
        ez4[0:64, 32 * b + 2 * b] = 1
        ez4[64:128, 32 * b + 2 * b + 1] = 1
    st16[:, ST16_EZ4:ST16_EZ4 + 128] = ez4

    st32 = np.zeros((128, ST32_COLS), np.float32)
    # ebc4: rows 32j..32j+8 = ebc pattern (z row 2hr+c -> oT chunk c parts 32hr)
    for jj in range(4):
        for c in range(2):
            for hr in range(4):
                st32[32 * jj + 2 * hr + c,
                     ST32_EBC + 128 * c + 32 * hr:ST32_EBC + 128 * c + 32 * (hr + 1)] = 1.0
    st32[:, ST32_BQ:ST32_BQ + 2] = np.stack([bqv[:128], bqv[128:]], axis=1)
    bpv = bv @ w_proj + b_proj
    st32[:, ST32_BP:ST32_BP + 2] = np.stack([bpv[:128], bpv[128:]], axis=1)

    return dict(st16=st16.astype(bfloat16), st32=st32.astype(np.float32))


TRACE = False
LAST_RESULT = None


def kernel(x, relative_bias, w_qkv, b_qkv, w_proj, b_proj):
    import sys
    if '/opt/trn_rl_repo' not in sys.path:
        sys.path.insert(0, '/opt/trn_rl_repo')
    import ml_dtypes
    from concourse.bass_utils import run_bass_kernel_spmd

    x = np.asarray(x, np.float32)
    const = _host_inputs(x,
                         np.asarray(relative_bias, np.float32),
                         np.asarray(w_qkv, np.float32),
                         np.asarray(b_qkv, np.float32),
                         np.asarray(w_proj, np.float32),
                         np.asarray(b_proj, np.float32))
    xr = np.ascontiguousarray(x.reshape(NBT, C, HW).astype(ml_dtypes.bfloat16))
    nc = _build_bass()
    in_maps = []
    for c in range(NCORES):
        m = dict(const)
        m["x"] = np.ascontiguousarray(xr[c * BT_PER_CORE:(c + 1) * BT_PER_CORE])
        in_maps.append(m)
    res = run_bass_kernel_spmd(nc, in_maps, list(range(NCORES)), trace=TRACE)
    global LAST_RESULT
    LAST_RESULT = res
    outs = res.results
    out = np.concatenate([o["out"].reshape(BT_PER_CORE, C, HW) for o in outs],
                         axis=0)
    return out.reshape(B, T, C, H, W).astype(np.float32)
